# revision 7
# baseline (speedup 1.0000x reference)
"""Trainium2 Bass kernel for nn_L3MParamHead.

Computes, for x[B,4096] @ W.T[4096,44] + b -> h[B,44]:
  params = sigmoid(h[:, :8])
  L (8x8 lower-tri) built from softplus(h[:,8:16])*s_i (diag) and
    h[:,16:44]*s_r (strict lower), s = 1/sqrt(0.25*arange(1,9))
  sigmas = L @ L.T            [B,8,8]
  logdet = log(softplus(1e4 * prod(diag(L))) / 1e4)

Strategy: pure data parallel over 8 cores (2048 rows each). Per core:
  - stream x in [128,4096] f32 slabs (2 MiB DMAs)
  - PE-transpose 128x128 blocks into PSUM (fp32r), copy to SBUF
  - matmul h.T[44,512] += WT_kc.T @ xT_kc (fp32r, N=512 -> 1 cyc/row)
  - bias/sigmoid/softplus/diag-scale applied in h.T layout ([44,512])
  - PE back-transpose to [128,44] row-major tiles for the per-row
    epilogue (L-flat build, LL^T via broadcast-AP products, logdet)
  - one fused [2048, 73] output per core: params | sigmas | logdet

The off-diagonal row scales s_r are folded into W/b on the host.
"""

import os
import sys

import numpy as np

sys.path.insert(0, "/opt/trn_rl_repo")

N_CORES = 8
B = 16384
HID = 4096
PD = 8
OUT = PD + (PD + 1) * PD // 2  # 44
# padded head layout (partition-32-aligned slices for ACT/DVE ops):
MP = 72        # padded M for the matmul / h.T partitions
DIAG0 = 32     # diag cols at 32:40
OFF0 = 40      # strict-lower cols at 40:68
B_SH = B // N_CORES  # 2048
GROUP_B = 512
GROUPS = B_SH // GROUP_B  # 4
KC = HID // 128  # 32
BETA = 10000.0
OUTW = PD + PD * PD + 1  # 73 fused output columns

_CACHE = {}


def _build_nc():
    from contextlib import ExitStack

    import concourse.bass as bass
    from concourse import bacc, mybir, tile

    f32 = mybir.dt.float32
    f32r = mybir.dt.float32r
    AF = mybir.ActivationFunctionType
    ALU = mybir.AluOpType

    nc = bacc.Bacc(target_bir_lowering=False, trn_type="TRN2")

    x_d = nc.dram_tensor("x", [B_SH, HID], f32r, kind="ExternalInput")
    wt_d = nc.dram_tensor("wt", [HID, MP], f32r, kind="ExternalInput")
    bias_d = nc.dram_tensor("bias", [MP, 1], f32, kind="ExternalInput")
    id128_d = nc.dram_tensor("id128", [128, 128], f32r, kind="ExternalInput")
    id72_d = nc.dram_tensor("id72", [MP, MP], f32, kind="ExternalInput")
    svec_d = nc.dram_tensor("svec", [MP, 1], f32, kind="ExternalInput")
    eye64_d = nc.dram_tensor("eye64", [128, PD * PD], f32, kind="ExternalInput")
    out_d = nc.dram_tensor("out", [B_SH, OUTW], f32, kind="ExternalOutput")

    with tile.TileContext(nc) as tc, ExitStack() as ctx:
        consts = ctx.enter_context(tc.tile_pool(name="consts", bufs=1))
        wt_pool = ctx.enter_context(tc.tile_pool(name="wt", bufs=1))
        slab_pool = ctx.enter_context(tc.tile_pool(name="slabs", bufs=8))
        xt_pool = ctx.enter_context(tc.tile_pool(name="xt", bufs=4))
        ht_pool = ctx.enter_context(tc.tile_pool(name="ht", bufs=2))
        ep_pool = ctx.enter_context(tc.tile_pool(name="ep", bufs=2))
        out_pool = ctx.enter_context(tc.tile_pool(name="outp", bufs=4))
        tp_psum = ctx.enter_context(tc.tile_pool(name="tp", bufs=4, space="PSUM"))
        acc_psum = ctx.enter_context(tc.tile_pool(name="acc", bufs=2, space="PSUM"))
        bt_psum = ctx.enter_context(tc.tile_pool(name="bt", bufs=1, space="PSUM"))

        id128 = consts.tile([128, 128], f32r)
        nc.sync.dma_start(id128[:], id128_d[:])
        id72 = consts.tile([MP, MP], f32)
        nc.sync.dma_start(id72[:], id72_d[:])
        svec = consts.tile([MP, 1], f32)
        nc.sync.dma_start(svec[:], svec_d[:])
        bias_t = consts.tile([MP, 1], f32)
        nc.sync.dma_start(bias_t[:], bias_d[:])
        eye64 = consts.tile([128, PD * PD], f32)
        nc.sync.dma_start(eye64[:], eye64_d[:])
        eye3 = eye64.rearrange("p (i j) -> p i j", i=PD)

        wt_t = wt_pool.tile([128, KC, MP], f32r)
        nc.sync.dma_start(wt_t[:], wt_d[:].rearrange("(c p) m -> p c m", p=128))

        for g in range(GROUPS):
            slabs = []
            for j in range(4):
                s = slab_pool.tile([128, HID], f32r, tag="slab")
                row0 = g * GROUP_B + j * 128
                nc.sync.dma_start(s[:], x_d[row0 : row0 + 128, :])
                slabs.append(s)

            acc = acc_psum.tile([MP, GROUP_B], f32, tag="acc")
            for kc in range(KC):
                tp = tp_psum.tile([128, GROUP_B], f32r, tag="tp")
                for j in range(4):
                    nc.tensor.transpose(
                        tp[:, j * 128 : (j + 1) * 128],
                        slabs[j][:, kc * 128 : (kc + 1) * 128],
                        id128[:],
                    )
                xt = xt_pool.tile([128, GROUP_B], f32r, tag="xt")
                nc.vector.tensor_copy(xt[:], tp[:])
                nc.tensor.matmul(
                    acc[:],
                    wt_t[:, kc, :],
                    xt[:],
                    start=(kc == 0),
                    stop=(kc == KC - 1),
                )

            # h.T epilogue: bias, sigmoid rows 0:8, softplus rows 32:40,
            # then whole-tile per-partition scale (svec = s_i at 32:40, 1 else)
            ht = ht_pool.tile([MP, GROUP_B], f32, tag="ht")
            nc.vector.tensor_scalar_add(ht[:], acc[:], bias_t[:, 0:1])
            nc.scalar.activation(ht[0:PD, :], ht[0:PD, :], AF.Sigmoid)
            # softplus = ln(1 + exp(x)) (h ~ N(0,1): no overflow risk)
            nc.scalar.activation(
                ht[DIAG0 : DIAG0 + PD, :], ht[DIAG0 : DIAG0 + PD, :], AF.Exp
            )
            nc.scalar.activation(
                ht[DIAG0 : DIAG0 + PD, :], ht[DIAG0 : DIAG0 + PD, :], AF.Ln, bias=1.0
            )
            nc.vector.tensor_scalar_mul(ht[:], ht[:], svec[:, 0:1])

            # back-transpose to row-major [128, 72] per 128-row subtile
            bt = bt_psum.tile([128, 4 * MP], f32, tag="bt")
            for j in range(4):
                nc.tensor.transpose(
                    bt[:, j * MP : (j + 1) * MP],
                    ht[:, j * 128 : (j + 1) * 128],
                    id72[:],
                )
            l44 = ep_pool.tile([128, 4 * MP], f32, tag="l44")
            nc.vector.tensor_copy(l44[:], bt[:])

            for j in range(4):
                row0 = g * GROUP_B + j * 128
                lsub = l44[:, j * MP : (j + 1) * MP]
                out_sb = out_pool.tile([128, OUTW], f32, tag="osb")

                # L-flat [128, 64]: diag via eye-mask product, off via copies
                lf = ep_pool.tile([128, PD * PD], f32, tag="lf")
                lf3 = lf.rearrange("p (i j) -> p i j", i=PD)
                ldb = (
                    lsub[:, DIAG0 : DIAG0 + PD]
                    .rearrange("p (i u) -> p i u", u=1)
                    .broadcast_to([128, PD, PD])
                )
                nc.vector.scalar_tensor_tensor(
                    lf3, ldb, 1.0, eye3, ALU.mult, ALU.mult
                )
                for r in range(1, PD):
                    src0 = OFF0 + r * (r - 1) // 2
                    nc.gpsimd.tensor_copy(
                        lf[:, PD * r : PD * r + r], lsub[:, src0 : src0 + r]
                    )

                # sigmas = sum_m A_m * B_m, split DVE (m<4) / GPSIMD (m>=4)
                def a_ap(m):
                    return lf3[:, :, m : m + 1].broadcast_to([128, PD, PD])

                def b_ap(m):
                    return (
                        lf3[:, :, m : m + 1]
                        .rearrange("p j u -> p u j")
                        .broadcast_to([128, PD, PD])
                    )

                sv = ep_pool.tile([128, PD * PD], f32, tag="sv")
                tv = ep_pool.tile([128, PD * PD], f32, tag="tv")
                sv3 = sv.rearrange("p (i j) -> p i j", i=PD)
                tv3 = tv.rearrange("p (i j) -> p i j", i=PD)
                nc.vector.tensor_mul(sv3, a_ap(0), b_ap(0))
                for m in range(1, 4):
                    nc.vector.tensor_mul(tv3, a_ap(m), b_ap(m))
                    nc.vector.tensor_add(sv3, sv3, tv3)
                sg = ep_pool.tile([128, PD * PD], f32, tag="sg")
                tg = ep_pool.tile([128, PD * PD], f32, tag="tg")
                sg3 = sg.rearrange("p (i j) -> p i j", i=PD)
                tg3 = tg.rearrange("p (i j) -> p i j", i=PD)
                nc.gpsimd.tensor_mul(sg3, a_ap(4), b_ap(4))
                for m in range(5, PD):
                    nc.gpsimd.tensor_mul(tg3, a_ap(m), b_ap(m))
                    nc.gpsimd.tensor_add(sg3, sg3, tg3)
                nc.vector.tensor_add(
                    out_sb[:, PD : PD + PD * PD], sv[:], sg[:]
                )

                # params passthrough (already sigmoid'd)
                nc.gpsimd.tensor_copy(out_sb[:, 0:PD], lsub[:, 0:PD])

                # logdet = ln(softplus(beta*p)/beta), p = prod(diag)
                # p = prod(diag) via pairwise muls
                t4 = ep_pool.tile([128, 4], f32, tag="t4")
                d = lsub[:, DIAG0 : DIAG0 + PD]
                nc.vector.tensor_mul(t4[:], d[:, 0:4], d[:, 4:8])
                t2 = ep_pool.tile([128, 2], f32, tag="t2")
                nc.vector.tensor_mul(t2[:], t4[:, 0:2], t4[:, 2:4])
                pr = ep_pool.tile([128, 1], f32, tag="pr")
                nc.vector.tensor_mul(pr[:], t2[:, 0:1], t2[:, 1:2])
                ex = ep_pool.tile([128, 1], f32, tag="ex")
                nc.scalar.activation(ex[:], pr[:], AF.Exp, scale=-BETA)
                lg = ep_pool.tile([128, 1], f32, tag="lg")
                nc.scalar.activation(lg[:], ex[:], AF.Ln, bias=1.0)
                spz = ep_pool.tile([128, 1], f32, tag="spz")
                nc.vector.scalar_tensor_tensor(
                    spz[:], pr[:], BETA, lg[:], ALU.mult, ALU.add
                )
                nc.scalar.activation(
                    out_sb[:, OUTW - 1 : OUTW], spz[:], AF.Ln, scale=1.0 / BETA
                )

                nc.sync.dma_start(out_d[row0 : row0 + 128, :], out_sb[:])

    nc.finalize()
    return nc


def _get_nc():
    if "nc" not in _CACHE:
        _CACHE["nc"] = _build_nc()
    return _CACHE["nc"]


def _host_inputs(x, W, b):
    s = (1.0 / np.sqrt(0.25 * np.arange(1, PD + 1))).astype(np.float32)
    r_off, _ = np.tril_indices(PD, -1)
    Wm = np.array(W, dtype=np.float32, copy=True)
    bm = np.array(b, dtype=np.float32, copy=True)
    Wm[2 * PD :] *= s[r_off][:, None]
    bm[2 * PD :] *= s[r_off]
    Wp = np.zeros((MP, HID), dtype=np.float32)
    bp = np.zeros(MP, dtype=np.float32)
    Wp[0:PD] = Wm[0:PD]
    Wp[DIAG0 : DIAG0 + PD] = Wm[PD : 2 * PD]
    Wp[OFF0 : OFF0 + OUT - 2 * PD] = Wm[2 * PD :]
    bp[0:PD] = bm[0:PD]
    bp[DIAG0 : DIAG0 + PD] = bm[PD : 2 * PD]
    bp[OFF0 : OFF0 + OUT - 2 * PD] = bm[2 * PD :]
    sv = np.ones(MP, dtype=np.float32)
    sv[DIAG0 : DIAG0 + PD] = s
    shared = {
        "wt": np.ascontiguousarray(Wp.T),
        "bias": np.ascontiguousarray(bp.reshape(MP, 1)),
        "id128": np.eye(128, dtype=np.float32),
        "id72": np.eye(MP, dtype=np.float32),
        "svec": np.ascontiguousarray(sv.reshape(MP, 1)),
        "eye64": np.ascontiguousarray(
            np.tile(np.eye(PD, dtype=np.float32).reshape(1, PD * PD), (128, 1))
        ),
    }
    x = np.asarray(x, dtype=np.float32)
    in_maps = []
    for i in range(N_CORES):
        m = dict(shared)
        m["x"] = np.ascontiguousarray(x[i * B_SH : (i + 1) * B_SH])
        in_maps.append(m)
    return in_maps


def _assemble(core_outs):
    full = np.concatenate([np.asarray(o["out"]) for o in core_outs], axis=0)
    params = np.ascontiguousarray(full[:, :PD], dtype=np.float32)
    sigmas = np.ascontiguousarray(full[:, PD : PD + PD * PD], dtype=np.float32)
    sigmas = sigmas.reshape(B, PD, PD)
    logdet = np.ascontiguousarray(full[:, OUTW - 1], dtype=np.float32)
    return params, sigmas, logdet


def run(inputs, trace=False):
    """Run on the 8 NeuronCores; returns (outputs_tuple, BassKernelResults)."""
    from concourse import bass_utils

    nc = _get_nc()
    in_maps = _host_inputs(inputs["x"], inputs["W"], inputs["b"])
    res = bass_utils.run_bass_kernel_spmd(
        nc, in_maps, core_ids=list(range(N_CORES)), trace=trace
    )
    return _assemble(res.results), res


def kernel(x, W, b):
    (params, sigmas, logdet), _ = run({"x": x, "W": W, "b": b})
    return params, sigmas, logdet
